# revision 7
# baseline (speedup 1.0000x reference)
"""Disentangled multi-head attention (DeBERTa-style) Trainium2 Bass kernel.

Full inputs in, full outputs out. Sharding: batch (B=8) across 8 cores, data
parallel; each core computes all H=8 heads for its batch element.

Math (per batch b):
  q,k,v = x@W? + b?                                   [S, D]
  P_k = rel_tab@Wpk + bpk ; P_q = rel_tab@Wpq + bpq   [1023, D]
  c2c[i,j] = q_i . k_j
  c2p[i,j] = q_i . P_k[j-i+511]  = qP[i, j-i+511],    qP  = q @ P_k^T
  p2c[i,j] = k_j . P_q[j-i+511]  = kPf[j, i-j+511],   kPf = k @ P_qflip^T
  out = softmax((c2c+c2p+p2c)/sqrt(3*64)) @ v ; y = out@Wo + bo

v2 design notes (cost-model-driven):
- All matmul operands bf16 (PE 1 cyc/row everywhere incl. the 128-wide
  window-tail matmuls; mixed 32/16-bit operands are rejected by neuronxcc).
  PSUM accumulation stays f32.
- Host marshals xT/relT pre-transposed and weights pre-cast to bf16: no PE
  transposes in phase A, and DRAM traffic is halved.
- Transposed-logits layout L[j, i] built in ONE 4-bank psum tile [128, 2048]:
  c2c matmuls + p2cT added via identity-matmul (I @ dgk) + c2pT via
  matmul-transpose trick (dgq_chunk^T @ I), all accumulating in psum. The
  separate DVE adds of the baseline are gone.
- qP/kPf window chunks go to ONE [128, 640] psum (2 banks), are evicted in a
  single op to a bf16 staging tile, and each pipeline's 4 chunks are pulled
  with ONE merged diagonal DMA (3-dim AP) -> 16 diag DMAs total instead of 64.
- DMA descriptor-generation is split between HWDGE (loads + qp diags, via
  SP) and the Pool engine's SWDGE (kp diags + recip broadcasts) so neither
  queue serializes the kernel.
- exp runs on ACT straight out of psum into a bf16 ex tile (two 1024-wide
  halves); softmax denominator via the ones-column AV trick; phase C stacks
  head pairs so output projection needs 16 matmuls (K=128).
"""

import math
import os
import sys
import threading

import ml_dtypes
import numpy as np

for _p in ("/opt/trn_rl_repo",):
    if _p not in sys.path and os.path.isdir(_p):
        sys.path.insert(0, _p)

import concourse.bacc as bacc
import concourse.bass as bass
import concourse.mybir as mybir
import concourse.tile as tile
from concourse.ap import AP
from concourse.bass_utils import run_bass_kernel_spmd
from concourse.masks import make_identity

S = 512
D = 512
H = 8
DH = 64
L = 512
W = 2 * L - 1  # 1023
WP = 1024
WIN = 640  # 639-wide diag window, rounded up
NCORES = 8
SCALE = 1.0 / math.sqrt(3.0 * DH)

F32 = mybir.dt.float32
BF16 = mybir.dt.bfloat16


def _diag_ap4(t, nrows, ncols):
    """Merged per-partition shifted read over 4 packed WIN-wide chunks:
    out[p, c*ncols + j] = t[p, c*WIN + 127 - p + j]."""
    rs = t.ap[0][0]
    return AP(
        t.tensor,
        t.offset + 127,
        [[rs - 1, nrows], [WIN, 4], [1, ncols]],
    )


def _rev_ap(t, ncols):
    """Free-dim reversed view of a [P, ncols] tile/psum AP."""
    rs = t.ap[0][0]
    return AP(t.tensor, t.offset + ncols - 1, [[rs, t.shape[0]], [-1, ncols]])


def build_program():
    nc = bacc.Bacc(trn_type="TRN2")

    xT = nc.dram_tensor("xT", [D, S], BF16, kind="ExternalInput")
    relT = nc.dram_tensor("relT", [D, WP], BF16, kind="ExternalInput")
    Wq = nc.dram_tensor("Wq", [D, D], BF16, kind="ExternalInput")
    Wk = nc.dram_tensor("Wk", [D, D], BF16, kind="ExternalInput")
    Wv = nc.dram_tensor("Wv", [D, D], BF16, kind="ExternalInput")
    Wpk = nc.dram_tensor("Wpk", [D, D], BF16, kind="ExternalInput")
    Wpq = nc.dram_tensor("Wpq", [D, D], BF16, kind="ExternalInput")
    Wo = nc.dram_tensor("Wo", [D, D], BF16, kind="ExternalInput")
    bcols = nc.dram_tensor("bcols", [128, 16], F32, kind="ExternalInput")
    bv = nc.dram_tensor("bv", [D], F32, kind="ExternalInput")
    bo = nc.dram_tensor("bo", [D], F32, kind="ExternalInput")
    y = nc.dram_tensor("y", [S, D], F32, kind="ExternalOutput")

    with tile.TileContext(nc) as tc:
        with (
            tc.tile_pool(name="const", bufs=1) as constp,
            tc.tile_pool(name="persist", bufs=1) as persist,
        ):
            ident_f = constp.tile([128, 128], F32, name="ident_f")
            make_identity(nc, ident_f)
            ident = constp.tile([128, 128], BF16, name="ident")
            nc.scalar.copy(ident[:], ident_f[:])

            def load_packed(dram, ncols, name, eng, pool):
                """One DMA: [512, ncols] dram -> [128, 4*ncols] sb tile with
                the four 128-row chunks packed along the free dim."""
                t = pool.tile([128, 4 * ncols], BF16, name=name)
                rs = t.ap[0][0]
                flat = dram[:, :].rearrange("a b -> (a b)")
                eng.dma_start(
                    AP(t.tensor, t.offset, [[rs, 128], [ncols, 4], [1, ncols]]),
                    AP(flat.tensor, 0, [[ncols, 128], [128 * ncols, 4], [1, ncols]]),
                )
                return [t[:, c * ncols : (c + 1) * ncols] for c in range(4)]

            # =========================== phase A ===========================
            with (
                tc.tile_pool(name="wload", bufs=1) as wload,
                tc.tile_pool(name="ps_prj", bufs=3, space="PSUM") as ps_prj,
                tc.tile_pool(name="ps_pos", bufs=2, space="PSUM") as ps_pos,
            ):
                xT_t = load_packed(xT, S, "xT", nc.sync, wload)
                Wq_t = load_packed(Wq, D, "Wq", nc.sync, wload)
                Wk_t = load_packed(Wk, D, "Wk", nc.sync, wload)
                relT_t = load_packed(relT, WP, "relT", nc.sync, wload)
                Wpk_t = load_packed(Wpk, D, "Wpk", nc.scalar, wload)
                Wpq_t = load_packed(Wpq, D, "Wpq", nc.scalar, wload)
                Wv_t = load_packed(Wv, D, "Wv", nc.scalar, wload)
                Wo_t = load_packed(Wo, D, "Wo", nc.sync, persist)

                bc_t = constp.tile([128, 16], F32, name="bc")
                nc.scalar.dma_start(bc_t[:], bcols[:, :])
                bv_bc = constp.tile([128, D], F32, name="bv_bc")
                nc.scalar.dma_start(
                    bv_bc[:], AP(bv[:].tensor, 0, [[0, 128], [1, D]])
                )
                bo_bc = constp.tile([128, D], F32, name="bo_bc")
                nc.scalar.dma_start(
                    bo_bc[:], AP(bo[:].tensor, 0, [[0, 128], [1, D]])
                )

                # ---- qT, kT: [d, s] bf16, per-partition bias ----
                def proj_T(W_t, bias_idx, name):
                    out = []
                    for dcc in range(4):
                        ps = ps_prj.tile([128, S], F32, name="ps_prj", tag="prj")
                        for ec in range(4):
                            nc.tensor.matmul(
                                ps[:],
                                W_t[ec][:, dcc * 128 : (dcc + 1) * 128],
                                xT_t[ec][:],
                                start=(ec == 0),
                                stop=(ec == 3),
                            )
                        t = persist.tile([128, S], BF16, name=f"{name}{dcc}")
                        nc.scalar.activation(
                            t[:],
                            ps[:],
                            mybir.ActivationFunctionType.Identity,
                            bias=bc_t[:, bias_idx * 4 + dcc : bias_idx * 4 + dcc + 1],
                        )
                        out.append(t)
                    return out

                qT_t = proj_T(Wq_t, 0, "qT")
                kT_t = proj_T(Wk_t, 1, "kT")

                # ---- v, written straight into ones-augmented head layout ----
                # vaug_sc[p, h*65 + c] = v[sc*128+p, h*64+c]; col h*65+64 = 1.
                vaug_t = []
                for sc in range(4):
                    ps = ps_prj.tile([128, D], F32, name="ps_v", tag="prj")
                    for ec in range(4):
                        nc.tensor.matmul(
                            ps[:],
                            xT_t[ec][:, sc * 128 : (sc + 1) * 128],
                            Wv_t[ec][:],
                            start=(ec == 0),
                            stop=(ec == 3),
                        )
                    va = persist.tile([128, H * (DH + 1)], BF16, name=f"vaug{sc}")
                    rs = va.ap[0][0]
                    nc.vector.tensor_add(
                        AP(va.tensor, va.offset, [[rs, 128], [DH + 1, H], [1, DH]]),
                        ps[:],
                        bv_bc[:],
                    )
                    nc.vector.memset(
                        AP(va.tensor, va.offset + DH, [[rs, 128], [DH + 1, H], [1, 1]]),
                        1.0,
                    )
                    vaug_t.append(va)

                # ---- P_kT [d, 1024] and P_qT flipped, bf16 ----
                PkT_t, PqTf_t = [], []
                for dcc in range(4):
                    for flip in (False, True):
                        W_t = Wpq_t if flip else Wpk_t
                        bias_idx = 3 if flip else 2
                        ps = ps_pos.tile([128, WP], F32, name="ps_pos", tag="pos")
                        for n0 in (0, 512):
                            for ec in range(4):
                                nc.tensor.matmul(
                                    ps[:, n0 : n0 + 512],
                                    W_t[ec][:, dcc * 128 : (dcc + 1) * 128],
                                    relT_t[ec][:, n0 : n0 + 512],
                                    start=(ec == 0),
                                    stop=(ec == 3),
                                )
                        t = persist.tile([128, WP], BF16, name=f"pos{flip}{dcc}")
                        b_ap = bc_t[:, bias_idx * 4 + dcc : bias_idx * 4 + dcc + 1]
                        if flip:
                            nc.scalar.activation(
                                t[:, 0:W],
                                _rev_ap(ps, W),
                                mybir.ActivationFunctionType.Identity,
                                bias=b_ap,
                            )
                            nc.vector.memset(t[:, W:WP], 0.0)
                            PqTf_t.append(t)
                        else:
                            nc.scalar.activation(
                                t[:],
                                ps[:],
                                mybir.ActivationFunctionType.Identity,
                                bias=b_ap,
                            )
                            PkT_t.append(t)

            # =========================== phase B ===========================
            with (
                tc.tile_pool(name="hwork", bufs=2) as hwork,
                tc.tile_pool(name="ps_win", bufs=2, space="PSUM") as ps_win,
                tc.tile_pool(name="ps_l", bufs=1, space="PSUM") as ps_l,
            ):
                evict_rr = [0]

                def qp_pipeline(thT, PhT, tag, diag_eng):
                    """4 window chunks -> one [128,640] psum each -> single-op
                    evict to a packed bf16 staging tile -> ONE merged diag DMA.
                    Returns dg [128, 2048] with chunk c at cols [c*512,(c+1)*512)."""
                    sb = hwork.tile([128, 4 * WIN], BF16, name=f"{tag}sb", bufs=2)
                    for ic in range(4):
                        i0 = ic * 128
                        pw = ps_win.tile([128, WIN], F32, name="ps_win", tag="win")
                        nc.tensor.matmul(
                            pw[:, 0:512],
                            thT[:, i0 : i0 + 128],
                            PhT[:, 384 - i0 : 896 - i0],
                        )
                        nc.tensor.matmul(
                            pw[:, 512:WIN],
                            thT[:, i0 : i0 + 128],
                            PhT[:, 896 - i0 : 1024 - i0],
                        )
                        r = evict_rr[0] % 3
                        evict_rr[0] += 1
                        dst = sb[:, ic * WIN : (ic + 1) * WIN]
                        if r == 0:
                            nc.vector.tensor_copy(dst, pw[:])
                        elif r == 1:
                            nc.scalar.copy(dst, pw[:])
                        else:
                            nc.gpsimd.tensor_copy(dst, pw[:])
                    dg = hwork.tile([128, 4 * S], BF16, name=f"{tag}dg", bufs=2)
                    rsd = dg.ap[0][0]
                    diag_eng.dma_start(
                        AP(dg.tensor, dg.offset, [[rsd, 128], [S, 4], [1, S]]),
                        _diag_ap4(sb, 128, S),
                    )
                    return dg

                def head_views(h):
                    dc, hs = h // 2, (h % 2) * DH
                    return (
                        qT_t[dc][hs : hs + DH, :],
                        kT_t[dc][hs : hs + DH, :],
                        PkT_t[dc][hs : hs + DH, :],
                        PqTf_t[dc][hs : hs + DH, :],
                    )

                def emit_pipes(h):
                    qhT, khT, PkhT, PqhTf = head_views(h)
                    dgq = qp_pipeline(qhT, PkhT, "qp", nc.sync)
                    dgk = qp_pipeline(khT, PqhTf, "kp", nc.gpsimd)
                    return dgq, dgk

                otp = [
                    persist.tile([128, S], BF16, name=f"otp{p}") for p in range(4)
                ]

                pipes = {0: emit_pipes(0)}
                for h in range(H):
                    qhT, khT, _, _ = head_views(h)
                    if h + 1 < H:
                        pipes[h + 1] = emit_pipes(h + 1)
                    dgq, dgk = pipes.pop(h)

                    # logits^T in one 4-bank psum tile: [j within jc, jc*512+i]
                    Lp = ps_l.tile([128, 4 * S], F32, name="Lp", tag="L")
                    for jc in range(4):
                        sl = Lp[:, jc * S : (jc + 1) * S]
                        nc.tensor.matmul(
                            sl,
                            khT[:, jc * 128 : (jc + 1) * 128],
                            qhT[:],
                            start=True,
                            stop=False,
                        )
                        nc.tensor.matmul(
                            sl,
                            ident[:],
                            dgk[:, jc * S : (jc + 1) * S],
                            start=False,
                            stop=False,
                        )
                        for ic in range(4):
                            nc.tensor.matmul(
                                Lp[:, jc * S + ic * 128 : jc * S + (ic + 1) * 128],
                                dgq[:, ic * S + jc * 128 : ic * S + (jc + 1) * 128],
                                ident[:],
                                start=False,
                                stop=(ic == 3),
                            )

                    et = hwork.tile([128, 4 * S], BF16, name="et", bufs=2)
                    for half in range(2):
                        nc.scalar.activation(
                            et[:, half * 1024 : (half + 1) * 1024],
                            Lp[:, half * 1024 : (half + 1) * 1024],
                            mybir.ActivationFunctionType.Exp,
                            scale=SCALE,
                        )

                    av = ps_win.tile([128, WIN], F32, name="ps_av", tag="win")
                    avv = av[0 : DH + 1, 0:S]
                    for jc in range(4):
                        nc.tensor.matmul(
                            avv,
                            vaug_t[jc][:, h * (DH + 1) : (h + 1) * (DH + 1)],
                            et[:, jc * S : (jc + 1) * S],
                            start=(jc == 0),
                            stop=(jc == 3),
                        )
                    zr = hwork.tile([1, S], F32, name="zr", bufs=2)
                    nc.vector.reciprocal(zr[:], av[DH : DH + 1, 0:S])
                    rb = hwork.tile([DH, S], F32, name="rb", bufs=2)
                    rsz = zr.ap[0][0]
                    nc.gpsimd.dma_start(
                        rb[:], AP(zr.tensor, zr.offset, [[rsz, 1], [0, DH], [1, S]])
                    )
                    nc.vector.tensor_mul(
                        otp[h // 2][(h % 2) * DH : (h % 2) * DH + DH, :],
                        av[0:DH, 0:S],
                        rb[:],
                    )

                # ======================= phase C ==========================
                ysb = hwork.tile([128, 4 * D], F32, name="ysb", bufs=1)
                psY = ps_l.tile([128, 4 * S], F32, name="psY", tag="L")
                for sc in range(4):
                    sl = psY[:, sc * D : (sc + 1) * D]
                    for p in range(4):
                        nc.tensor.matmul(
                            sl,
                            otp[p][:, sc * 128 : (sc + 1) * 128],
                            Wo_t[p][:],
                            start=(p == 0),
                            stop=(p == 3),
                        )
                    nc.vector.tensor_add(
                        ysb[:, sc * D : (sc + 1) * D], sl, bo_bc[:]
                    )
                rsy = ysb.ap[0][0]
                yflat = y[:, :].rearrange("a b -> (a b)")
                nc.sync.dma_start(
                    AP(yflat.tensor, 0, [[D, 128], [128 * D, 4], [1, D]]),
                    AP(ysb.tensor, ysb.offset, [[rsy, 128], [D, 4], [1, D]]),
                )

    nc.compile()
    return nc


_cache_lock = threading.Lock()
_cached_nc = None


def _get_program():
    global _cached_nc
    with _cache_lock:
        if _cached_nc is None:
            _cached_nc = build_program()
    return _cached_nc


def kernel(**inputs):
    x = np.ascontiguousarray(np.asarray(inputs["x"], dtype=np.float32))
    B = x.shape[0]
    assert x.shape == (B, S, D)
    bf = ml_dtypes.bfloat16

    relT = np.zeros((D, WP), dtype=bf)
    relT[:, :W] = np.asarray(inputs["rel_tab"], dtype=np.float32).T

    def w_bf(name):
        return np.ascontiguousarray(
            np.asarray(inputs[name], dtype=np.float32).astype(bf)
        )

    def b_col(name):
        return np.asarray(inputs[name], dtype=np.float32).reshape(4, 128).T

    weights = {
        "relT": relT,
        "Wq": w_bf("Wq"),
        "Wk": w_bf("Wk"),
        "Wv": w_bf("Wv"),
        "Wpk": w_bf("Wpk"),
        "Wpq": w_bf("Wpq"),
        "Wo": w_bf("Wo"),
        "bcols": np.ascontiguousarray(
            np.concatenate(
                [b_col(n) for n in ("bq", "bk", "bpk", "bpq")], axis=1
            ).astype(np.float32)
        ),
        "bv": np.asarray(inputs["bv"], dtype=np.float32),
        "bo": np.asarray(inputs["bo"], dtype=np.float32),
    }

    nc = _get_program()
    in_maps = [
        {"xT": np.ascontiguousarray(x[c].T.astype(bf)), **weights}
        for c in range(NCORES)
    ]
    res = run_bass_kernel_spmd(nc, in_maps, core_ids=list(range(NCORES)))
    out = np.stack([res.results[c]["y"] for c in range(NCORES)], axis=0)
    return out.astype(np.float32)


if __name__ == "__main__":
    rng = np.random.default_rng(0)
    ins = {
        "x": rng.standard_normal((NCORES, S, D), dtype=np.float32),
        "rel_tab": rng.standard_normal((W, D), dtype=np.float32),
    }
    for nm in ("Wq", "Wk", "Wv", "Wpk", "Wpq", "Wo"):
        ins[nm] = rng.standard_normal((D, D), dtype=np.float32) * 0.04
    for nm in ("bq", "bk", "bv", "bpk", "bpq", "bo"):
        ins[nm] = rng.standard_normal(D).astype(np.float32) * 0.01
    out = kernel(**ins)
    print("ran:", out.shape, out.dtype, np.abs(out).max())
